# revision 1
# baseline (speedup 1.0000x reference)
"""LRU single-step kernel for 8x TRN2 NeuronCores (Bass/Tile).

Math (per batch row b, hidden h):
  out_re[b,h] = lam_re[h]*h_re[b,h] - lam_im[h]*h_im[b,h] + (x @ (scale*B_real).T)[b,h]
  out_im[b,h] = lam_im[h]*h_re[b,h] + lam_re[h]*h_im[b,h] + (x @ (scale*B_img ).T)[b,h]

Strategy: data-parallel over the batch axis (8 shards of 32768 rows). On each
core, everything is computed in a transposed layout (hidden on partitions,
batch on the free axis) so that the Lambda elementwise terms become diagonal-
weight matmuls accumulating into the same PSUM tile as the input projection:

  psum_re[h,b] = W_re[i,h].T @ x_t[i,b] + diag(lam_re) @ hre_t[h,b] + diag(-lam_im) @ him_t[h,b]

This keeps DVE/ACT down to PSUM->SBUF copies; the kernel is DMA-bound.
Host-side prep (shard + transpose of the big tensors, tiny param math) is
done in numpy.

PE Matmult instructions only have one sync-wait slot in codegen, so waits are
carefully absorbed before real matmuls run:
  - per-iteration 1x1 "lane absorber" matmuls read one freshly-DMA'd tile each
    (and write a persistent scratch PSUM tile), so each carries exactly one
    DMA-lane wait and advances the PE's observed clock;
  - PSUM tiles are allocated once and reused manually (no pool recycling), so
    no TileRelease edges exist on PSUM: the first matmul of a group carries
    only the WAR wait on the previous iteration's PSUM->SBUF copy.
"""

import numpy as np

import concourse.bass as bass
import concourse.mybir as mybir
from concourse.tile import TileContext
from concourse.bass_utils import run_bass_kernel_spmd

B_SZ, IN_DIM, HID = 262144, 128, 256
N_CORES = 8
S = B_SZ // N_CORES  # 32768 rows per core
P = 128
HCHUNKS = HID // P  # 2
COLS = 1024          # batch columns per outer iteration
OUTER = S // COLS    # 32
MMF = 512            # matmul free dim (one fp32 PSUM bank)
NBLK = COLS // MMF   # 2

# consts layout (one (128, 1280) f32 tensor):
#   [:, 0:256]     w_re  = (scale*B_real).T
#   [:, 256:512]   w_im  = (scale*B_img).T
#   [:, 512:768]   diag(lam_re)  chunks 0,1
#   [:, 768:1024]  diag(lam_im)  chunks 0,1
#   [:, 1024:1280] diag(-lam_im) chunks 0,1
CONST_COLS = 1280

F32 = mybir.dt.float32

_cache = {}

# Stashed BassKernelResults from the most recent run (for test harnesses).
LAST_RESULTS = None


def _build():
    if "nc" in _cache:
        return _cache["nc"]

    nc = bass.Bass(trn_type="TRN2")

    x_t = nc.dram_tensor("x_t", (IN_DIM, S), F32, kind="ExternalInput")
    hre_t = nc.dram_tensor("hre_t", (HID, S), F32, kind="ExternalInput")
    him_t = nc.dram_tensor("him_t", (HID, S), F32, kind="ExternalInput")
    consts = nc.dram_tensor("consts", (P, CONST_COLS), F32, kind="ExternalInput")

    o_re = nc.dram_tensor("o_re", (HID, S), F32, kind="ExternalOutput")
    o_im = nc.dram_tensor("o_im", (HID, S), F32, kind="ExternalOutput")

    hre_v = hre_t[:, :].rearrange("(c p) s -> p c s", p=P)
    him_v = him_t[:, :].rearrange("(c p) s -> p c s", p=P)
    ore_v = o_re[:, :].rearrange("(c p) s -> p c s", p=P)
    oim_v = o_im[:, :].rearrange("(c p) s -> p c s", p=P)

    with TileContext(nc) as tc:
        with (
            tc.tile_pool(name="cpool", bufs=1) as cpool,
            tc.tile_pool(name="xin", bufs=3) as xin,
            tc.tile_pool(name="hin", bufs=3) as hin,
            tc.tile_pool(name="outp", bufs=3) as outp,
            tc.tile_pool(name="psum", bufs=1, space="PSUM") as psum,
        ):
            csb = cpool.tile([P, CONST_COLS], F32)
            nc.gpsimd.dma_start(csb[:], consts[:, :])
            # 7 persistent data PSUM tiles + 1 scratch; allocated once so no
            # TileRelease/realloc wait sets ever form on PSUM.
            ps_tiles = [psum.tile([P, MMF], F32, tag=f"ps{i}", name=f"ps{i}")
                        for i in range(7)]
            scratch = psum.tile([P, 8], F32, tag="scratch")
            _cache["ps_idx"] = 0

            def lane_absorb(tile_ap):
                # 1x1 matmul reading the freshly-DMA'd tile: carries exactly
                # one DMA-lane wait, advancing the PE's observed clock so the
                # real matmuls don't re-wait on that lane.
                nc.tensor.matmul(scratch[0:1, 0:1], tile_ap, tile_ap,
                                 start=True, stop=True, skip_group_check=True)

            w_re_sb = csb[:, 0:HID]
            w_im_sb = csb[:, HID:2 * HID]

            def dre_c(c):
                return csb[:, 2 * HID + c * P: 2 * HID + (c + 1) * P]

            def dim_c(c):
                return csb[:, 3 * HID + c * P: 3 * HID + (c + 1) * P]

            def dimn_c(c):
                return csb[:, 4 * HID + c * P: 4 * HID + (c + 1) * P]

            lane_absorb(csb[0:1, 0:1])

            for o in range(OUTER):
                sl = slice(o * COLS, (o + 1) * COLS)
                xt = xin.tile([P, COLS], F32)
                nc.gpsimd.dma_start(xt[:], x_t[:, sl])
                hre = hin.tile([P, HCHUNKS, COLS], F32, tag="hre")
                him = hin.tile([P, HCHUNKS, COLS], F32, tag="him")
                nc.gpsimd.dma_start(hre[:], hre_v[:, :, sl])
                nc.gpsimd.dma_start(him[:], him_v[:, :, sl])
                lane_absorb(xt[0:1, 0:1])
                lane_absorb(hre[0:1, 0, 0:1])
                lane_absorb(him[0:1, 0, 0:1])

                ore = outp.tile([P, HCHUNKS, COLS], F32, tag="ore")
                oim = outp.tile([P, HCHUNKS, COLS], F32, tag="oim")

                copy_i = 0
                for c in range(HCHUNKS):
                    wre_c = w_re_sb[:, c * P:(c + 1) * P]
                    wim_c = w_im_sb[:, c * P:(c + 1) * P]
                    for b in range(NBLK):
                        bs = slice(b * MMF, (b + 1) * MMF)
                        xs = xt[:, bs]
                        hres = hre[:, c, bs]
                        hims = him[:, c, bs]

                        ps_re = ps_tiles[_cache["ps_idx"] % 7]
                        _cache["ps_idx"] += 1
                        nc.tensor.matmul(ps_re[:], wre_c, xs, start=True, stop=False)
                        nc.tensor.matmul(ps_re[:], dre_c(c), hres, start=False, stop=False)
                        nc.tensor.matmul(ps_re[:], dimn_c(c), hims, start=False, stop=True)

                        ps_im = ps_tiles[_cache["ps_idx"] % 7]
                        _cache["ps_idx"] += 1
                        nc.tensor.matmul(ps_im[:], wim_c, xs, start=True, stop=False)
                        nc.tensor.matmul(ps_im[:], dim_c(c), hres, start=False, stop=False)
                        nc.tensor.matmul(ps_im[:], dre_c(c), hims, start=False, stop=True)

                        # One engine per output tile so each store DMA waits
                        # on a single semaphore: ore <- ACT, oim <- DVE.
                        nc.scalar.copy(ore[:, c, bs], ps_re[:])
                        nc.vector.tensor_copy(oim[:, c, bs], ps_im[:])
                        copy_i += 1

                nc.gpsimd.dma_start(ore_v[:, :, sl], ore[:])
                nc.gpsimd.dma_start(oim_v[:, :, sl], oim[:])

    _split_multiwaits(nc)
    _cache["nc"] = nc
    return nc


def _split_multiwaits(nc):
    """walrus codegen allows exactly one semaphore wait per instruction.
    Move all-but-one wait of every multi-wait instruction onto single-wait
    NOP instructions spliced immediately before it on the same engine
    (engines execute their stream in order, so semantics are unchanged)."""
    k = 0
    for bb in nc.m.functions[0].blocks:
        new_list = []
        for ins in bb.instructions:
            si = ins.sync_info
            if si is not None and si.on_wait and len(si.on_wait) > 1:
                for w in si.on_wait[:-1]:
                    nop = mybir.InstNoOp(
                        name=f"WN-{k}", engine=ins.engine,
                        sync_info=mybir.SyncInfo(on_wait=[w], on_update=[]),
                    )
                    k += 1
                    new_list.append(nop)
                si.on_wait = [si.on_wait[-1]]
            new_list.append(ins)
        bb.instructions[:] = new_list


def kernel(inputs, h_re, h_im, nu_log, theta_log, B_real, B_img, gamma_log):
    global LAST_RESULTS
    inputs = np.asarray(inputs, dtype=np.float32)
    h_re = np.asarray(h_re, dtype=np.float32)
    h_im = np.asarray(h_im, dtype=np.float32)
    nu_log = np.asarray(nu_log, dtype=np.float32)
    theta_log = np.asarray(theta_log, dtype=np.float32)
    B_real = np.asarray(B_real, dtype=np.float32)
    B_img = np.asarray(B_img, dtype=np.float32)
    gamma_log = np.asarray(gamma_log, dtype=np.float32)

    # Tiny parameter math on host (matches the f32 reference computation).
    mag = np.exp(-np.exp(nu_log))          # (1, H)
    theta = np.exp(theta_log)              # (1, H)
    lam_re = (mag * np.cos(theta))[0]      # (H,)
    lam_im = (mag * np.sin(theta))[0]      # (H,)
    scale = np.exp(gamma_log).T            # (H, 1)
    w_re = (scale * B_real).T              # (IN_DIM, H)
    w_im = (scale * B_img).T               # (IN_DIM, H)

    consts = np.zeros((P, CONST_COLS), np.float32)
    consts[:, 0:HID] = w_re
    consts[:, HID:2 * HID] = w_im
    idx = np.arange(P)
    for c in range(HCHUNKS):
        lr = lam_re[c * P:(c + 1) * P]
        li = lam_im[c * P:(c + 1) * P]
        consts[idx, 2 * HID + c * P + idx] = lr
        consts[idx, 3 * HID + c * P + idx] = li
        consts[idx, 4 * HID + c * P + idx] = -li

    in_maps = []
    for core in range(N_CORES):
        sl = slice(core * S, (core + 1) * S)
        in_maps.append({
            "x_t": np.ascontiguousarray(inputs[sl].T),
            "hre_t": np.ascontiguousarray(h_re[sl].T),
            "him_t": np.ascontiguousarray(h_im[sl].T),
            "consts": consts,
        })

    nc = _build()
    res = run_bass_kernel_spmd(nc, in_maps, core_ids=list(range(N_CORES)))
    LAST_RESULTS = res

    out = np.empty((2, B_SZ, HID), np.float32)
    for core in range(N_CORES):
        sl = slice(core * S, (core + 1) * S)
        out[0, sl] = res.results[core]["o_re"].T
        out[1, sl] = res.results[core]["o_im"].T
    return out



# revision 2
# speedup vs baseline: 3.8720x; 3.8720x over previous
"""LRU single-step kernel for 8x TRN2 NeuronCores (Bass/Tile), bf16 datapath.

Math (per batch row b, hidden h):
  out_re[b,h] = lam_re[h]*h_re[b,h] - lam_im[h]*h_im[b,h] + (x @ (scale*B_real).T)[b,h]
  out_im[b,h] = lam_im[h]*h_re[b,h] + lam_re[h]*h_im[b,h] + (x @ (scale*B_img ).T)[b,h]

Strategy: data-parallel over the batch axis (8 shards of 32768 rows), all
HBM traffic in bf16 (the 2e-2 rel-err budget gives ~8x slack over bf16
quantization noise, and the problem is memory-bound: 144 MiB/core in f32
halves to 72 MiB/core in bf16).

On each core everything is computed in a transposed layout (hidden on
partitions, batch on the free axis). The 256 hiddens are split into 4 groups
of 64; for group g the SBUF/PSUM partition layout packs re and im halves
together:  partitions 0:64 <- h_re[g*64:(g+1)*64], 64:128 <- h_im[...].
With that packing each output tile needs exactly TWO matmuls accumulated in
PSUM instead of six:

  psum[j, b]    = Wp_g[i, j].T    @ x_t[i, b]      (proj_re | proj_im packed)
                + Wlam_g[p, j].T  @ hcat_g[p, b]   (block-diagonal lambda mix)

where Wp_g  = [w_re cols g | w_im cols g]  (128 x 128, bf16)
      Wlam_g = [[diag(lam_re)  diag(lam_im)]
                [diag(-lam_im) diag(lam_re)]]    (128 x 128, bf16)

x is carried as a 5th partition-group of the same input tensor, so each
outer iteration is ONE load DMA (gpsimd/SWDGE), 32 bf16 matmuls, 16
PSUM->SBUF downcasting copies (split ACT/DVE), and ONE store DMA issued on
the Sync engine (HWDGE) so stores never block load descriptor generation.

PE Matmult instructions only have one sync-wait slot in codegen, so waits
are absorbed before real matmuls run (1x1 "lane absorber" matmuls per DMA'd
tile + persistent manually-rotated PSUM tiles); _split_multiwaits moves any
remaining multi-waits onto NOPs.
"""

import numpy as np

import concourse.bass as bass
import concourse.mybir as mybir
from concourse.tile import TileContext
from concourse.bass_utils import run_bass_kernel_spmd

B_SZ, IN_DIM, HID = 262144, 128, 256
N_CORES = 8
S = B_SZ // N_CORES     # 32768 rows per core
P = 128
NGRP = HID // 64        # 4 hidden groups of 64 (re+im packed per group)
GIN = NGRP + 1          # input tensor has 4 h-groups + 1 x-group
COLS = 2048             # batch columns per outer iteration
OUTER = S // COLS       # 16
MMF = 512               # matmul free dim (one fp32 PSUM bank)
NBLK = COLS // MMF      # 4

# consts layout: (128, 1024) bf16; for g in 0..3:
#   [:, g*256      : g*256+128]  Wp_g   = [w_re[:, g] | w_im[:, g]]
#   [:, g*256+128  : g*256+256]  Wlam_g (block-diagonal lambda mixer)
CONST_COLS = NGRP * 256

F32 = mybir.dt.float32
BF16 = mybir.dt.bfloat16
NP_BF16 = mybir.dt.np(mybir.dt.bfloat16)

_cache = {}

# Stashed BassKernelResults from the most recent run (for test harnesses).
LAST_RESULTS = None


def _build():
    if "nc" in _cache:
        return _cache["nc"]

    nc = bass.Bass(trn_type="TRN2")

    hx = nc.dram_tensor("hx", (GIN * P, S), BF16, kind="ExternalInput")
    consts = nc.dram_tensor("consts", (P, CONST_COLS), BF16, kind="ExternalInput")
    ocat = nc.dram_tensor("ocat", (NGRP * P, S), BF16, kind="ExternalOutput")

    hx_v = hx[:, :].rearrange("(g p) s -> p g s", p=P)
    oc_v = ocat[:, :].rearrange("(g p) s -> p g s", p=P)

    with TileContext(nc) as tc:
        with (
            tc.tile_pool(name="cpool", bufs=1) as cpool,
            tc.tile_pool(name="hxin", bufs=3) as hxin,
            tc.tile_pool(name="outp", bufs=3) as outp,
            tc.tile_pool(name="psum", bufs=1, space="PSUM") as psum,
        ):
            csb = cpool.tile([P, CONST_COLS], BF16)
            nc.gpsimd.dma_start(csb[:], consts[:, :])
            # 7 persistent data PSUM tiles + 1 scratch; allocated once so no
            # TileRelease/realloc wait sets ever form on PSUM.
            ps_tiles = [psum.tile([P, MMF], F32, tag=f"ps{i}", name=f"ps{i}")
                        for i in range(7)]
            scratch = psum.tile([P, 8], F32, tag="scratch")
            _cache["ps_idx"] = 0

            def lane_absorb(tile_ap):
                # 1x1 matmul reading the freshly-DMA'd tile: carries exactly
                # one DMA-lane wait, advancing the PE's observed clock so the
                # real matmuls don't re-wait on that lane.
                nc.tensor.matmul(scratch[0:1, 0:1], tile_ap, tile_ap,
                                 start=True, stop=True, skip_group_check=True)

            def wp_g(g):
                return csb[:, g * 256: g * 256 + 128]

            def wlam_g(g):
                return csb[:, g * 256 + 128: g * 256 + 256]

            lane_absorb(csb[0:1, 0:1])

            for o in range(OUTER):
                sl = slice(o * COLS, (o + 1) * COLS)
                ht = hxin.tile([P, GIN, COLS], BF16, tag="ht")
                nc.gpsimd.dma_start(ht[:], hx_v[:, :, sl])
                lane_absorb(ht[0:1, 0, 0:1])

                ot = outp.tile([P, NGRP, COLS], BF16, tag="ot")

                ci = 0
                for g in range(NGRP):
                    for b in range(NBLK):
                        bs = slice(b * MMF, (b + 1) * MMF)
                        ps = ps_tiles[_cache["ps_idx"] % 7]
                        _cache["ps_idx"] += 1
                        nc.tensor.matmul(ps[:], wp_g(g), ht[:, NGRP, bs],
                                         start=True, stop=False)
                        nc.tensor.matmul(ps[:], wlam_g(g), ht[:, g, bs],
                                         start=False, stop=True)
                        # Alternate PSUM->SBUF copy engines so each runs at
                        # half the tile rate: even -> ACT, odd -> DVE.
                        if ci % 2 == 0:
                            nc.scalar.copy(ot[:, g, bs], ps[:])
                        else:
                            nc.vector.tensor_copy(ot[:, g, bs], ps[:])
                        ci += 1

                # Store on the Sync engine (HWDGE): keeps store descriptor
                # generation off the GpSimd queue so loads prefetch freely.
                nc.sync.dma_start(oc_v[:, :, sl], ot[:])

    _split_multiwaits(nc)
    _cache["nc"] = nc
    return nc


def _split_multiwaits(nc):
    """walrus codegen allows exactly one semaphore wait per instruction.
    Move all-but-one wait of every multi-wait instruction onto single-wait
    NOP instructions spliced immediately before it on the same engine
    (engines execute their stream in order, so semantics are unchanged)."""
    k = 0
    for bb in nc.m.functions[0].blocks:
        new_list = []
        for ins in bb.instructions:
            si = ins.sync_info
            if si is not None and si.on_wait and len(si.on_wait) > 1:
                for w in si.on_wait[:-1]:
                    nop = mybir.InstNoOp(
                        name=f"WN-{k}", engine=ins.engine,
                        sync_info=mybir.SyncInfo(on_wait=[w], on_update=[]),
                    )
                    k += 1
                    new_list.append(nop)
                si.on_wait = [si.on_wait[-1]]
            new_list.append(ins)
        bb.instructions[:] = new_list


def kernel(inputs, h_re, h_im, nu_log, theta_log, B_real, B_img, gamma_log):
    global LAST_RESULTS
    inputs = np.asarray(inputs, dtype=np.float32)
    h_re = np.asarray(h_re, dtype=np.float32)
    h_im = np.asarray(h_im, dtype=np.float32)
    nu_log = np.asarray(nu_log, dtype=np.float32)
    theta_log = np.asarray(theta_log, dtype=np.float32)
    B_real = np.asarray(B_real, dtype=np.float32)
    B_img = np.asarray(B_img, dtype=np.float32)
    gamma_log = np.asarray(gamma_log, dtype=np.float32)

    # Tiny parameter math on host (matches the f32 reference computation).
    mag = np.exp(-np.exp(nu_log))          # (1, H)
    theta = np.exp(theta_log)              # (1, H)
    lam_re = (mag * np.cos(theta))[0]      # (H,)
    lam_im = (mag * np.sin(theta))[0]      # (H,)
    scale = np.exp(gamma_log).T            # (H, 1)
    w_re = (scale * B_real).T              # (IN_DIM, H)
    w_im = (scale * B_img).T               # (IN_DIM, H)

    consts = np.zeros((P, CONST_COLS), np.float32)
    j = np.arange(64)
    for g in range(NGRP):
        base = g * 256
        hs = slice(g * 64, (g + 1) * 64)
        consts[:, base: base + 64] = w_re[:, hs]
        consts[:, base + 64: base + 128] = w_im[:, hs]
        lr = lam_re[hs]
        li = lam_im[hs]
        # Wlam_g[p, jj]: out col jj<64 is re, jj>=64 is im.
        consts[j, base + 128 + j] = lr
        consts[64 + j, base + 128 + j] = -li
        consts[j, base + 192 + j] = li
        consts[64 + j, base + 192 + j] = lr
    consts = consts.astype(NP_BF16)

    in_maps = []
    for core in range(N_CORES):
        sl = slice(core * S, (core + 1) * S)
        hx = np.empty((GIN * P, S), NP_BF16)
        hv = hx[: NGRP * P].reshape(NGRP, 2, 64, S)
        hv[:, 0] = h_re[sl].astype(NP_BF16).reshape(S, NGRP, 64).transpose(1, 2, 0)
        hv[:, 1] = h_im[sl].astype(NP_BF16).reshape(S, NGRP, 64).transpose(1, 2, 0)
        hx[NGRP * P:] = inputs[sl].astype(NP_BF16).T
        in_maps.append({"hx": hx, "consts": consts})

    nc = _build()
    res = run_bass_kernel_spmd(nc, in_maps, core_ids=list(range(N_CORES)))
    LAST_RESULTS = res

    out = np.empty((2, B_SZ, HID), np.float32)
    for core in range(N_CORES):
        sl = slice(core * S, (core + 1) * S)
        oc = res.results[core]["ocat"].reshape(NGRP, 2, 64, S).astype(np.float32)
        # out[r, s, g*64 + j] = oc[g, r, j, s]
        out[:, sl, :] = oc.transpose(1, 3, 0, 2).reshape(2, S, HID)
    return out
